# revision 2
# baseline (speedup 1.0000x reference)
"""2-layer GAT (nn_GATNet) on 8 TRN2 NeuronCores — self-contained kernel.

Architecture (SPMD, one program on 8 cores, dst-node sharding 6250/core):
  phase A1 (replicated, fp16): hext1[n] = [x@W1 | x@Wa1s | x@Wa1d] for all
      nodes -> DRAM table [N, 80] fp16 (80-elem rows, 160B stride). The local
      shard's rows are also kept in SBUF (hx1) for self-loop terms.
  phase B1 (edge phase): edges sorted by dst, sharded by dst range; per
      128-dst window, T tiles of 128 edge slots (uniform schedule across
      cores; padded slots use src=0 with a one-hot offset that matches
      nothing). Per window: ONE batched indirect-DMA gather of hext1[src]
      rows ([128, kT, 72], multi-column offset AP), plus a narrow batched
      gather of alpha_dst by dst id (element_offset into the same table,
      7 windows per instruction). logits -> leaky_relu -> exp; segment
      softmax realized as U = sum(ex*h), denom = sum(ex) accumulated in PSUM
      by one-hot matmuls (one-hot built by vector is_equal, fp16), then
      agg = U/denom. Softmax max-subtraction omitted (ratio-invariant;
      logits bounded).
  phase A2: h2 = elu(h1)@W_ext2 for the local shard -> AllGather -> hext2
      [N, 42] fp16 table.
  phase B2: same edge phase with 1 head / 40 dims, then log_softmax, output
      shard [6250, 40] fp32; host concatenates shards.
"""
import numpy as np
import ml_dtypes
import concourse.bass as bass
import concourse.bacc as bacc
import concourse.tile as tile
from concourse import mybir
from concourse.bass_utils import run_bass_kernel_spmd

P = 128
F32 = mybir.dt.float32
F16 = mybir.dt.float16
I32 = mybir.dt.int32
AF = mybir.ActivationFunctionType
OP = mybir.AluOpType
PADOFF = 200.0

N_NODES = 50000
NC = 8
WB = 2      # dst-windows per main gather
KAD = 7     # dst-windows per alpha_dst gather


def _fold_params(W1, a1_src, a1_dst, W2, a2_src, a2_dst):
    def fold(W, a):
        heads, od = a.shape
        return np.einsum("cho,ho->ch", W.reshape(W.shape[0], heads, od), a)
    W_ext1 = np.concatenate([W1, fold(W1, a1_src), fold(W1, a1_dst)], axis=1)
    W_ext2 = np.concatenate([W2, fold(W2, a2_src), fold(W2, a2_dst)], axis=1)
    return (np.ascontiguousarray(W_ext1, ml_dtypes.float16),
            np.ascontiguousarray(W_ext2, ml_dtypes.float16))


def _prep_edges(src, dst, N, T):
    """Per core: slot (p, col=w*T+t) holds edge j = t*128+p of window w."""
    shard = N // NC
    NW = (shard + P - 1) // P
    NT = NW * T
    per_core = []
    for c in range(NC):
        lo = c * shard
        m = (dst >= lo) & (dst < lo + shard)
        s_c = src[m].astype(np.int64)
        d_c = dst[m].astype(np.int64)
        ld = d_c - lo
        order = np.argsort(ld, kind="stable")
        s_c, d_c, ld = s_c[order], d_c[order], ld[order]
        win = ld >> 7
        off = (ld & 127).astype(np.float32)
        src_idx = np.zeros((NT, P), np.int32)
        dstg_idx = np.full((NT, P), lo, np.int32)
        dstoff = np.full((NT, P), PADOFF, np.float32)
        wstart = np.searchsorted(win, np.arange(NW + 1))
        for w in range(NW):
            a, b = wstart[w], wstart[w + 1]
            cnt = b - a
            assert cnt <= T * P, f"window overflow: {cnt} > {T * P}"
            blk = src_idx[w * T:(w + 1) * T]
            blk.reshape(-1)[:cnt] = s_c[a:b]
            dstg_idx[w * T:(w + 1) * T].reshape(-1)[:cnt] = d_c[a:b]
            dstoff[w * T:(w + 1) * T].reshape(-1)[:cnt] = off[a:b]
        per_core.append((np.ascontiguousarray(src_idx.T),
                         np.ascontiguousarray(dstg_idx.T),
                         np.ascontiguousarray(dstoff.T.astype(ml_dtypes.float16))))
    return per_core, NW, NT


def build_kernel(N, T, reps=1):
    shard = N // NC
    NW = (shard + P - 1) // P
    NT = NW * T
    NTA = (N + P - 1) // P
    W1O, W2O = 80, 42     # fp16 table row widths (h | a_src | a_dst)
    G1, G2 = 72, 41       # gathered row prefix (h | a_src)
    CH = 16

    nc = bacc.Bacc("TRN2", target_bir_lowering=False, debug=False)

    xT = nc.dram_tensor("xT", [P, N], F16, kind="ExternalInput")
    W_ext1 = nc.dram_tensor("W_ext1", [P, W1O], F16, kind="ExternalInput")
    W_ext2 = nc.dram_tensor("W_ext2", [64, W2O], F16, kind="ExternalInput")
    b1m = nc.dram_tensor("b1m", [P, 64], F32, kind="ExternalInput")
    b2m = nc.dram_tensor("b2m", [P, 40], F32, kind="ExternalInput")
    iota_in = nc.dram_tensor("iota_in", [P, P], F16, kind="ExternalInput")
    ident_in = nc.dram_tensor("ident_in", [P, P], F16, kind="ExternalInput")
    src_idx = nc.dram_tensor("src_idx", [P, NT], I32, kind="ExternalInput")
    dstg_in = nc.dram_tensor("dstg_in", [P, NT], I32, kind="ExternalInput")
    doff_in = nc.dram_tensor("doff_in", [P, NT], F16, kind="ExternalInput")
    out = nc.dram_tensor("out", [shard, 40], F32, kind="ExternalOutput")

    hext1 = nc.dram_tensor("hext1", [N, W1O], F16)
    h2_shard = nc.dram_tensor("h2_shard", [shard, W2O], F16)
    hext2 = nc.dram_tensor("hext2", [N, W2O], F16, addr_space="Shared")

    lo = None  # per-core dst range start is implicit via input data

    with tile.TileContext(nc) as tc:
        cp = tc.alloc_tile_pool(name="const", bufs=1)
        w1_sb = cp.tile([P, W1O], F16)
        nc.sync.dma_start(out=w1_sb[:], in_=W_ext1[:])
        w2_sb = cp.tile([64, W2O], F16)
        nc.sync.dma_start(out=w2_sb[:], in_=W_ext2[:])
        b1_sb = cp.tile([P, 64], F32)
        nc.sync.dma_start(out=b1_sb[:], in_=b1m[:])
        b2_sb = cp.tile([P, 40], F32)
        nc.sync.dma_start(out=b2_sb[:], in_=b2m[:])
        iota_sb = cp.tile([P, P], F16)
        nc.sync.dma_start(out=iota_sb[:], in_=iota_in[:])
        ident_sb = cp.tile([P, P], F16)
        nc.sync.dma_start(out=ident_sb[:], in_=ident_in[:])
        sidx_sb = cp.tile([P, NT], I32)
        nc.sync.dma_start(out=sidx_sb[:], in_=src_idx[:])
        dstg_sb = cp.tile([P, NT], I32)
        nc.sync.dma_start(out=dstg_sb[:], in_=dstg_in[:])
        doff_sb = cp.tile([P, NT], F16)
        nc.sync.dma_start(out=doff_sb[:], in_=doff_in[:])

        hx1 = cp.tile([P, NW, W1O], F16)      # local shard rows of hext1
        hx2 = cp.tile([P, NW, W2O], F16)      # local shard rows of hext2
        h1act = cp.tile([P, NW, 64], F16)     # elu(h1) local
        uall = cp.tile([P, NW, 72], F32)      # edge-phase U accumulators
        ual2 = cp.tile([P, NW, 41], F32)

        gp = tc.alloc_tile_pool(name="gp", bufs=3)
        ap_ = tc.alloc_tile_pool(name="adp", bufs=2)
        ohp = tc.alloc_tile_pool(name="ohp", bufs=2)
        rp = tc.alloc_tile_pool(name="rp", bufs=2)
        sp = tc.alloc_tile_pool(name="sp", bufs=3)
        fp = tc.alloc_tile_pool(name="fin", bufs=1)
        pu = tc.alloc_tile_pool(name="pu", bufs=2, space="PSUM")

        def gather(dest_ap, table, idx_ap, element_offset=0):
            nc.gpsimd.indirect_dma_start(
                out=dest_ap, out_offset=None, in_=table[:],
                in_offset=bass.IndirectOffsetOnAxis(ap=idx_ap, axis=0),
                element_offset=element_offset)

        def edge_phase(table, WROW, GROW, NH, OD, uacc):
            HC = NH * OD
            for w0 in range(0, NW, WB):
                nw = min(WB, NW - w0)
                if w0 % KAD == 0:
                    nad = min(KAD, NW - w0)
                    ad_b = ap_.tile([P, KAD * T, NH], F16, tag="ad_b")
                    gather(ad_b[:, 0:nad * T, :], table,
                           dstg_sb[:, w0 * T:(w0 + nad) * T],
                           element_offset=HC + NH)
                    ad_cur = ad_b
                    adw0 = w0
                g_b = gp.tile([P, WB * T, GROW], F16, tag="g_b")
                gather(g_b[:, 0:nw * T, :], table,
                       sidx_sb[:, w0 * T:(w0 + nw) * T])
                for wi in range(nw):
                    w = w0 + wi
                    gw = g_b[:, wi * T:(wi + 1) * T, :]
                    adw = ad_cur[:, (w - adw0) * T:(w - adw0 + 1) * T, :]
                    e_b = sp.tile([P, T, NH], F16, tag="e_b")
                    nc.vector.tensor_add(out=e_b[:], in0=gw[:, :, HC:HC + NH],
                                         in1=adw)
                    l_b = sp.tile([P, T, NH], F16, tag="l_b")
                    nc.vector.scalar_tensor_tensor(
                        out=l_b[:], in0=e_b[:], scalar=0.2, in1=e_b[:],
                        op0=OP.mult, op1=OP.max)
                    rhs_b = rp.tile([P, T, HC + NH], F16, tag="rhs_b")
                    nc.scalar.activation(out=rhs_b[:, :, HC:HC + NH],
                                         in_=l_b[:], func=AF.Exp)
                    nc.vector.tensor_tensor(
                        out=rhs_b[:, :, 0:HC].rearrange(
                            "p s (h o) -> p s h o", o=OD),
                        in0=gw[:, :, 0:HC].rearrange("p s (h o) -> p s h o", o=OD),
                        in1=rhs_b[:, :, HC:HC + NH, None].to_broadcast(
                            [P, T, NH, OD]),
                        op=OP.mult)
                    oh_b = ohp.tile([P, T, P], F16, tag="oh_b")
                    nc.vector.tensor_tensor(
                        out=oh_b[:],
                        in0=doff_sb[:, w * T:(w + 1) * T, None].to_broadcast(
                            [P, T, P]),
                        in1=iota_sb[:, None, :].to_broadcast([P, T, P]),
                        op=OP.is_equal)
                    U_ps = pu.tile([P, HC + NH], F32, space="PSUM", tag="U")
                    for t in range(T):
                        nc.tensor.matmul(
                            out=U_ps[:], lhsT=oh_b[:, t, :], rhs=rhs_b[:, t, :],
                            start=(t == 0), stop=(t == T - 1))
                    nc.scalar.activation(out=uacc[:, w, 0:HC + NH], in_=U_ps[:],
                                         func=AF.Copy)

        def finish(hx, uacc, NH, OD, bias):
            """Self-loop terms + normalize -> agg [P, NW, HC] f32 (+bias)."""
            HC = NH * OD
            es = fp.tile([P, NW, NH], F16, tag="es")
            nc.vector.tensor_add(out=es[:], in0=hx[:, :, HC:HC + NH],
                                 in1=hx[:, :, HC + NH:HC + 2 * NH])
            ls = fp.tile([P, NW, NH], F16, tag="ls")
            nc.vector.scalar_tensor_tensor(
                out=ls[:], in0=es[:], scalar=0.2, in1=es[:],
                op0=OP.mult, op1=OP.max)
            exs = fp.tile([P, NW, NH], F32, tag="exs")
            nc.scalar.activation(out=exs[:], in_=ls[:], func=AF.Exp)
            exs16 = fp.tile([P, NW, NH], F16, tag="exs16")
            nc.vector.tensor_copy(out=exs16[:], in_=exs[:])
            den = fp.tile([P, NW, NH], F32, tag="den")
            nc.vector.tensor_add(out=den[:], in0=uacc[:, :, HC:HC + NH],
                                 in1=exs[:])
            Uf = fp.tile([P, NW, HC], F32, tag="Uf")
            nc.vector.tensor_tensor(
                out=Uf[:].rearrange("p w (h o) -> p w h o", o=OD),
                in0=hx[:, :, 0:HC].rearrange("p w (h o) -> p w h o", o=OD),
                in1=exs16[:, :, :, None].to_broadcast([P, NW, NH, OD]),
                op=OP.mult)
            nc.vector.tensor_add(out=Uf[:], in0=Uf[:], in1=uacc[:, :, 0:HC])
            recip = fp.tile([P, NW, NH], F32, tag="recip")
            nc.vector.reciprocal(recip[:], den[:])
            agg = fp.tile([P, NW, HC], F32, tag="agg")
            nc.vector.tensor_tensor(
                out=agg[:].rearrange("p w (h o) -> p w h o", o=OD),
                in0=Uf[:].rearrange("p w (h o) -> p w h o", o=OD),
                in1=recip[:, :, :, None].to_broadcast([P, NW, NH, OD]),
                op=OP.mult)
            nc.vector.tensor_add(
                out=agg[:], in0=agg[:],
                in1=bias[:, None, :].to_broadcast([P, NW, HC]))
            return agg

        for rep in range(reps):
            # ---- phase A1: hext1 = [x@W1 | x@Wa1s | x@Wa1d], replicated ----
            with (tc.tile_pool(name="xa", bufs=2) as xa,
                  tc.tile_pool(name="ha", bufs=2) as ha,
                  tc.tile_pool(name="pa", bufs=2, space="PSUM") as pa):
                for ch in range(0, NTA, CH):
                    ntile = min(CH, NTA - ch)
                    cols = min(CH * P, N - ch * P)
                    xc = xa.tile([P, CH * P], F16, tag="xc")
                    nc.sync.dma_start(out=xc[:, :cols],
                                      in_=xT[:, ch * P:ch * P + cols])
                    hb = ha.tile([P, CH, W1O], F16, tag="hb")
                    for t in range(ntile):
                        n0 = (ch + t) * P
                        rows = min(P, N - n0)
                        ps = pa.tile([P, W1O], F32, space="PSUM", tag="psA")
                        nc.tensor.matmul(out=ps[:rows, :],
                                         lhsT=xc[:, t * P:t * P + rows],
                                         rhs=w1_sb[:], start=True, stop=True)
                        nc.scalar.activation(out=hb[:rows, t, :],
                                             in_=ps[:rows, :], func=AF.Copy)
                    rows_ch = min(CH * P, N - ch * P)
                    nc.sync.dma_start(
                        out=hext1[ch * P:ch * P + rows_ch, :],
                        in_=hb[:].rearrange("p t w -> (t p) w")[0:rows_ch, :])
                # local shard rows -> SBUF copy (reload from DRAM, one DMA)
            with tc.tile_pool(name="lx", bufs=1) as lx:
                loc = lx.tile([P, NW, W1O], F16, tag="loc")
                nc.partition_broadcast  # noqa: B018  (placeholder no-op attr)
                del loc
            # (hx1 filled by DMA below)
            nc.sync.dma_start(
                out=hx1[:].rearrange("p w c -> (w p) c")[0:shard, :],
                in_=hext1[0:shard, :])  # placeholder, fixed per-core below

            # ---- phase B1 ----
            edge_phase(hext1, W1O, G1, 8, 8, uall)
            agg1 = finish(hx1, uall, 8, 8, b1_sb)
            ex1 = fp.tile([P, NW, 64], F32, tag="ex1")
            nc.scalar.activation(out=ex1[:], in_=agg1[:], func=AF.Exp)
            em = fp.tile([P, NW, 64], F32, tag="em")
            nc.vector.tensor_scalar(out=em[:], in0=ex1[:], scalar1=-1.0,
                                    scalar2=0.0, op0=OP.add, op1=OP.min)
            nc.vector.scalar_tensor_tensor(
                out=h1act[:], in0=agg1[:], scalar=0.0, in1=em[:],
                op0=OP.max, op1=OP.add)

            # ---- phase A2: h2 = elu(h1) @ W_ext2, local shard + AllGather ----
            with (tc.tile_pool(name="a2", bufs=3) as a2,
                  tc.tile_pool(name="p2", bufs=2, space="PSUM") as p2):
                for w in range(NW):
                    rows = min(P, shard - w * P)
                    hT_ps = p2.tile([64, P], F32, space="PSUM", tag="hT")
                    nc.tensor.transpose(out=hT_ps[:], in_=h1act[:, w, :],
                                        identity=ident_sb[:])
                    hT_sb = a2.tile([64, P], F16, tag="hT_sb")
                    nc.scalar.activation(out=hT_sb[:], in_=hT_ps[:], func=AF.Copy)
                    ps2 = p2.tile([P, W2O], F32, space="PSUM", tag="ps2")
                    nc.tensor.matmul(out=ps2[:], lhsT=hT_sb[:], rhs=w2_sb[:],
                                     start=True, stop=True)
                    nc.scalar.activation(out=hx2[:, w, :], in_=ps2[:],
                                         func=AF.Copy)
                nc.sync.dma_start(
                    out=h2_shard[:],
                    in_=hx2[:].rearrange("p w c -> (w p) c")[0:shard, :])
            nc.gpsimd.collective_compute(
                "AllGather", OP.bypass, replica_groups=[list(range(NC))],
                ins=[h2_shard[:]], outs=[hext2[:]])

            # ---- phase B2 ----
            edge_phase(hext2, W2O, G2, 1, 40, ual2)
            agg2 = finish(hx2, ual2, 1, 40, b2_sb)
            mx = fp.tile([P, NW, 1], F32, tag="mx")
            nc.vector.reduce_max(out=mx[:], in_=agg2[:], axis=mybir.AxisListType.X)
            tm = fp.tile([P, NW, 40], F32, tag="tm")
            nc.vector.tensor_sub(out=tm[:], in0=agg2[:],
                                 in1=mx[:, :, 0:1].to_broadcast([P, NW, 40]))
            q = fp.tile([P, NW, 40], F32, tag="q")
            nc.scalar.activation(out=q[:], in_=tm[:], func=AF.Exp)
            s = fp.tile([P, NW, 1], F32, tag="s")
            nc.vector.reduce_sum(out=s[:], in_=q[:], axis=mybir.AxisListType.X)
            lsf = fp.tile([P, NW, 1], F32, tag="lsf")
            nc.scalar.activation(out=lsf[:], in_=s[:], func=AF.Ln)
            o = fp.tile([P, NW, 40], F32, tag="o")
            nc.vector.tensor_sub(out=o[:], in0=tm[:],
                                 in1=lsf[:, :, 0:1].to_broadcast([P, NW, 40]))
            nc.sync.dma_start(
                out=out[:],
                in_=o[:].rearrange("p w c -> (w p) c")[0:shard, :])

        for pool in (pu, fp, sp, rp, ohp, ap_, gp, cp):
            pool.release()

    nc.compile()
    return nc


# revision 27
# speedup vs baseline: 1.2008x; 1.2008x over previous
"""2-layer GAT (nn_GATNet) on 8 TRN2 NeuronCores — self-contained kernel.

Architecture (SPMD, one program on 8 cores, dst-node sharding 6250/core):
  phase A1 (replicated, fp16): hext1[n] = [x@W1 | x@Wa1s | x@Wa1d] for all
      nodes -> DRAM table [N, 80] fp16. Per window, one [P,1] indirect
      gather pulls the window's 128 dst rows into SBUF (hx1) for self-loop
      terms and the alpha_dst column.
  phase B1 (edge phase): edges sorted by dst, sharded by dst range; per
      128-dst window, T tiles of 128 edge slots (uniform schedule across
      cores; padded slots use src=0 with a one-hot offset matching
      nothing). Per tile, one [P,1] indirect gather of hext1[src] rows
      (fp16, 144B/edge). alpha_dst per edge = ohT @ ad_window where ohT
      (transposed one-hot of dst offsets) is built on-device: a K=1
      outer-product matmul replicates the window's dst-offset stream
      across partitions, then vector is_equal against an iota column.
      logits -> leaky_relu -> exp; segment softmax as U = sum(ex*h),
      denom = sum(ex), accumulated in PSUM by one-hot matmuls (one-hot
      via vector is_equal, fp16); agg = U/denom. Softmax max-subtraction
      omitted (ratio-invariant; logits bounded).
  phase A2: h2 = elu(h1)@W_ext2 for the local shard, AllGather -> hext2
      [N, 42] fp16 table.
  phase B2: same edge phase with 1 head / 40 dims, then log_softmax, output
      shard [6250, 40] fp32; host concatenates shards.
"""
import numpy as np
import concourse.bass as bass
import concourse.bacc as bacc
import concourse.tile as tile
from concourse import mybir
from concourse.bass_utils import run_bass_kernel_spmd

P = 128
F32 = mybir.dt.float32
F16 = mybir.dt.float16
I32 = mybir.dt.int32
AF = mybir.ActivationFunctionType
OP = mybir.AluOpType
PADOFF = 200.0

N_NODES = 50000
NC = 8
OHT_MODE = "rep"   # "rep" (K=1 outer-product) or "pe" (PE transpose of oh)


def _fold_params(W1, a1_src, a1_dst, W2, a2_src, a2_dst):
    def fold(W, a):
        heads, od = a.shape
        return np.einsum("cho,ho->ch", W.reshape(W.shape[0], heads, od), a)
    W_ext1 = np.concatenate([W1, fold(W1, a1_src), fold(W1, a1_dst)], axis=1)
    W_ext2 = np.concatenate([W2, fold(W2, a2_src), fold(W2, a2_dst)], axis=1)
    return (np.ascontiguousarray(W_ext1, np.float16),
            np.ascontiguousarray(W_ext2, np.float16))


def _prep_edges(src, dst, N, T, n_cores):
    """Per core: slot (p, col=w*T+t) holds edge j = t*128+p of window w."""
    shard = N // n_cores
    NW = (shard + P - 1) // P
    NT = NW * T
    per_core = []
    for c in range(n_cores):
        lo = c * shard
        m = (dst >= lo) & (dst < lo + shard)
        s_c = src[m].astype(np.int64)
        ld = (dst[m] - lo).astype(np.int64)
        order = np.argsort(ld, kind="stable")
        s_c, ld = s_c[order], ld[order]
        win = ld >> 7
        off = (ld & 127).astype(np.float32)
        src_idx = np.zeros((NT, P), np.int32)
        dstoff = np.full((NT, P), PADOFF, np.float32)
        wstart = np.searchsorted(win, np.arange(NW + 1))
        for w in range(NW):
            a, b = wstart[w], wstart[w + 1]
            cnt = b - a
            assert cnt <= T * P, f"window overflow: {cnt} > {T * P}"
            src_idx[w * T:(w + 1) * T].reshape(-1)[:cnt] = s_c[a:b]
            dstoff[w * T:(w + 1) * T].reshape(-1)[:cnt] = off[a:b]
        doffT = np.zeros((NW, T * P), np.float16)
        for w in range(NW):
            doffT[w] = dstoff[w * T:(w + 1) * T].reshape(-1).astype(np.float16)
        ids = lo + np.arange(NW * P)
        ids[ids >= lo + shard] = lo
        dwin = ids.reshape(NW, P).astype(np.int32)
        per_core.append((np.ascontiguousarray(src_idx.T),
                         np.ascontiguousarray(dstoff.T.astype(np.float16)),
                         np.ascontiguousarray(doffT),
                         np.ascontiguousarray(dwin.T)))
    return per_core, NW, NT


def build_kernel(N, T, reps=1, n_cores=NC):
    shard = N // n_cores
    NW = (shard + P - 1) // P
    NT = NW * T
    NTA = (N + P - 1) // P
    CH = 16
    TC = T * P
    W1O, W2O = 80, 42
    G1, G2 = 72, 41

    nc = bacc.Bacc("TRN2", target_bir_lowering=False, debug=False)

    xT = nc.dram_tensor("xT", [P, N], F16, kind="ExternalInput")
    W_ext1 = nc.dram_tensor("W_ext1", [P, W1O], F16, kind="ExternalInput")
    W_ext2 = nc.dram_tensor("W_ext2", [64, W2O], F16, kind="ExternalInput")
    b1m = nc.dram_tensor("b1m", [P, 64], F32, kind="ExternalInput")
    b2m = nc.dram_tensor("b2m", [P, 40], F32, kind="ExternalInput")
    iota_in = nc.dram_tensor("iota_in", [P, P], F16, kind="ExternalInput")
    iotac_in = nc.dram_tensor("iotac_in", [P, 1], F32, kind="ExternalInput")
    ones_in = nc.dram_tensor("ones_in", [1, P], F16, kind="ExternalInput")
    ident_in = nc.dram_tensor("ident_in", [P, P], F16, kind="ExternalInput")
    sidx_in = nc.dram_tensor("sidx_in", [P, NT], I32, kind="ExternalInput")
    doff_in = nc.dram_tensor("doff_in", [P, NT], F16, kind="ExternalInput")
    dofT_in = nc.dram_tensor("dofT_in", [NW, TC], F16, kind="ExternalInput")
    dwin_in = nc.dram_tensor("dwin_in", [P, NW], I32, kind="ExternalInput")
    out = nc.dram_tensor("out", [shard, 40], F32, kind="ExternalOutput")

    hext1 = nc.dram_tensor("hext1", [N, W1O], F16)
    h2_shard = nc.dram_tensor("h2_shard", [shard, W2O], F16)
    hext2 = nc.dram_tensor("hext2", [N, W2O], F16, addr_space="Shared")

    with tile.TileContext(nc) as tc:
        cp = tc.alloc_tile_pool(name="const", bufs=1)
        w1_sb = cp.tile([P, W1O], F16)
        nc.sync.dma_start(out=w1_sb[:], in_=W_ext1[:])
        w2_sb = cp.tile([64, W2O], F16)
        nc.sync.dma_start(out=w2_sb[:], in_=W_ext2[:])
        b1_sb = cp.tile([P, 64], F32)
        nc.sync.dma_start(out=b1_sb[:], in_=b1m[:])
        b2_sb = cp.tile([P, 40], F32)
        nc.sync.dma_start(out=b2_sb[:], in_=b2m[:])
        iota_sb = cp.tile([P, P], F16)
        nc.sync.dma_start(out=iota_sb[:], in_=iota_in[:])
        iotac_sb = cp.tile([P, 1], F32)
        nc.sync.dma_start(out=iotac_sb[:], in_=iotac_in[:])
        ones_sb = cp.tile([1, P], F16)
        nc.sync.dma_start(out=ones_sb[:], in_=ones_in[:])
        ident_sb = cp.tile([P, P], F16)
        nc.sync.dma_start(out=ident_sb[:], in_=ident_in[:])
        sidx_sb = cp.tile([P, NT], I32)
        nc.sync.dma_start(out=sidx_sb[:], in_=sidx_in[:])
        doff_sb = cp.tile([P, NT], F16)
        nc.sync.dma_start(out=doff_sb[:], in_=doff_in[:])
        dwin_sb = cp.tile([P, NW], I32)
        nc.sync.dma_start(out=dwin_sb[:], in_=dwin_in[:])

        hx1 = cp.tile([P, NW, W1O], F16)     # local dst rows of hext1
        hx2 = cp.tile([P, NW, W2O], F16)     # local rows of hext2 (A2 out)
        h1act = cp.tile([P, NW, 64], F16)    # elu(h1) local
        uall = cp.tile([P, NW, 72], F32)
        ual2 = cp.tile([P, NW, 41], F32)

        gp = tc.alloc_tile_pool(name="gp", bufs=3)
        ohp = tc.alloc_tile_pool(name="ohp", bufs=2)
        rp = tc.alloc_tile_pool(name="rp", bufs=2)
        sp = tc.alloc_tile_pool(name="sp", bufs=3)
        dfp = tc.alloc_tile_pool(name="dfp", bufs=2)
        fp = tc.alloc_tile_pool(name="fin", bufs=1)
        pu = tc.alloc_tile_pool(name="pu", bufs=2, space="PSUM")
        pr = tc.alloc_tile_pool(name="pr", bufs=1, space="PSUM")
        pd = tc.alloc_tile_pool(name="pd", bufs=2, space="PSUM")

        def gather1(dest_ap, table, idx_col):
            nc.gpsimd.indirect_dma_start(
                out=dest_ap, out_offset=None, in_=table[:],
                in_offset=bass.IndirectOffsetOnAxis(ap=idx_col, axis=0))

        def edge_phase(table, GROW, NH, OD, uacc, hx):
            HC = NH * OD
            for w in range(NW):
                g_b = gp.tile([P, T, GROW], F16, tag="g_b")
                for t in range(T):
                    gather1(g_b[:, t, :], table,
                            sidx_sb[:, w * T + t:w * T + t + 1])
                # transposed one-hot [off, slot] for all T tiles
                ohT = ohp.tile([P, TC], F16, tag="ohT")
                if OHT_MODE == "rep":
                    dofT_w = dfp.tile([1, TC], F16, tag="dofT_w")
                    nc.sync.dma_start(out=dofT_w[:], in_=dofT_in[w:w + 1, :])
                    for c0 in range(0, TC, 512):
                        cn = min(512, TC - c0)
                        rg = pr.tile([P, 512], F32, space="PSUM", tag="rep")
                        nc.tensor.matmul(out=rg[:, 0:cn], lhsT=ones_sb[:],
                                         rhs=dofT_w[:, c0:c0 + cn],
                                         start=True, stop=True)
                        nc.vector.tensor_tensor(
                            out=ohT[:, c0:c0 + cn],
                            in0=iotac_sb[:, 0:1].to_broadcast([P, cn]),
                            in1=rg[:, 0:cn], op=OP.is_equal)
                oh_b = ohp.tile([P, T, P], F16, tag="oh_b")
                nc.vector.tensor_tensor(
                    out=oh_b[:],
                    in0=doff_sb[:, w * T:(w + 1) * T, None].to_broadcast(
                        [P, T, P]),
                    in1=iota_sb[:, None, :].to_broadcast([P, T, P]),
                    op=OP.is_equal)
                if OHT_MODE == "pe":
                    for t in range(T):
                        tp = pr.tile([P, P], F32, space="PSUM", tag="rep")
                        nc.tensor.transpose(out=tp[:], in_=oh_b[:, t, :],
                                            identity=ident_sb[:])
                        nc.scalar.activation(out=ohT[:, t * P:(t + 1) * P],
                                             in_=tp[:], func=AF.Copy)
                ade = pd.tile([P, T, NH], F32, space="PSUM", tag="ade")
                for t in range(T):
                    nc.tensor.matmul(
                        out=ade[:, t, :], lhsT=ohT[:, t * P:(t + 1) * P],
                        rhs=hx[:, w, HC + NH:HC + 2 * NH],
                        start=True, stop=True)
                ade16 = sp.tile([P, T, NH], F16, tag="ade16")
                nc.scalar.activation(out=ade16[:], in_=ade[:], func=AF.Copy)
                e_b = sp.tile([P, T, NH], F16, tag="e_b")
                nc.vector.tensor_add(out=e_b[:], in0=g_b[:, :, HC:HC + NH],
                                     in1=ade16[:])
                l_b = sp.tile([P, T, NH], F16, tag="l_b")
                nc.vector.scalar_tensor_tensor(
                    out=l_b[:], in0=e_b[:], scalar=0.2, in1=e_b[:],
                    op0=OP.mult, op1=OP.max)
                rhs_b = rp.tile([P, T, HC + NH], F16, tag="rhs_b")
                nc.scalar.activation(out=rhs_b[:, :, HC:HC + NH], in_=l_b[:],
                                     func=AF.Exp)
                nc.vector.tensor_tensor(
                    out=rhs_b[:, :, 0:HC].rearrange("p s (h o) -> p s h o",
                                                    o=OD),
                    in0=g_b[:, :, 0:HC].rearrange("p s (h o) -> p s h o", o=OD),
                    in1=rhs_b[:, :, HC:HC + NH, None].to_broadcast(
                        [P, T, NH, OD]),
                    op=OP.mult)
                U_ps = pu.tile([P, HC + NH], F32, space="PSUM", tag="U")
                for t in range(T):
                    nc.tensor.matmul(
                        out=U_ps[:], lhsT=oh_b[:, t, :], rhs=rhs_b[:, t, :],
                        start=(t == 0), stop=(t == T - 1))
                nc.scalar.activation(out=uacc[:, w, 0:HC + NH], in_=U_ps[:],
                                     func=AF.Copy)

        def finish(hx, uacc, NH, OD, bias):
            HC = NH * OD
            es = fp.tile([P, NW, NH], F16, tag="es")
            nc.vector.tensor_add(out=es[:], in0=hx[:, :, HC:HC + NH],
                                 in1=hx[:, :, HC + NH:HC + 2 * NH])
            ls = fp.tile([P, NW, NH], F16, tag="ls")
            nc.vector.scalar_tensor_tensor(
                out=ls[:], in0=es[:], scalar=0.2, in1=es[:],
                op0=OP.mult, op1=OP.max)
            exs = fp.tile([P, NW, NH], F32, tag="exs")
            nc.scalar.activation(out=exs[:], in_=ls[:], func=AF.Exp)
            exs16 = fp.tile([P, NW, NH], F16, tag="exs16")
            nc.vector.tensor_copy(out=exs16[:], in_=exs[:])
            den = fp.tile([P, NW, NH], F32, tag="den")
            nc.vector.tensor_add(out=den[:], in0=uacc[:, :, HC:HC + NH],
                                 in1=exs[:])
            Uf = fp.tile([P, NW, HC], F32, tag="Uf")
            nc.vector.tensor_tensor(
                out=Uf[:].rearrange("p w (h o) -> p w h o", o=OD),
                in0=hx[:, :, 0:HC].rearrange("p w (h o) -> p w h o", o=OD),
                in1=exs16[:, :, :, None].to_broadcast([P, NW, NH, OD]),
                op=OP.mult)
            nc.vector.tensor_add(out=Uf[:], in0=Uf[:], in1=uacc[:, :, 0:HC])
            recip = fp.tile([P, NW, NH], F32, tag="recip")
            nc.vector.reciprocal(recip[:], den[:])
            agg = fp.tile([P, NW, HC], F32, tag="agg")
            nc.vector.tensor_tensor(
                out=agg[:].rearrange("p w (h o) -> p w h o", o=OD),
                in0=Uf[:].rearrange("p w (h o) -> p w h o", o=OD),
                in1=recip[:, :, :, None].to_broadcast([P, NW, NH, OD]),
                op=OP.mult)
            nc.vector.tensor_add(
                out=agg[:], in0=agg[:],
                in1=bias[:, None, :].to_broadcast([P, NW, HC]))
            return agg

        def dma_rows_out(dram, start, nrows, sb3d):
            full = nrows // P
            if full:
                nc.sync.dma_start(
                    out=dram[start:start + full * P, :].rearrange(
                        "(t p) w -> p t w", p=P),
                    in_=sb3d[:, 0:full, :])
            rem = nrows - full * P
            if rem:
                nc.sync.dma_start(out=dram[start + full * P:start + nrows, :],
                                  in_=sb3d[0:rem, full, :])

        for rep in range(reps):
            # ---- phase A1 ----
            with (tc.tile_pool(name="xa", bufs=2) as xa,
                  tc.tile_pool(name="ha", bufs=2) as ha,
                  tc.tile_pool(name="pa", bufs=2, space="PSUM") as pa):
                for ch in range(0, NTA, CH):
                    ntile = min(CH, NTA - ch)
                    cols = min(CH * P, N - ch * P)
                    xc = xa.tile([P, CH * P], F16, tag="xc")
                    nc.sync.dma_start(out=xc[:, :cols],
                                      in_=xT[:, ch * P:ch * P + cols])
                    hb = ha.tile([P, CH, W1O], F16, tag="hb")
                    for t in range(ntile):
                        n0 = (ch + t) * P
                        rows = min(P, N - n0)
                        ps = pa.tile([P, W1O], F32, space="PSUM", tag="psA")
                        nc.tensor.matmul(out=ps[:rows, :],
                                         lhsT=xc[:, t * P:t * P + rows],
                                         rhs=w1_sb[:], start=True, stop=True)
                        nc.scalar.activation(out=hb[:rows, t, :],
                                             in_=ps[:rows, :], func=AF.Copy)
                    dma_rows_out(hext1, ch * P, min(CH * P, N - ch * P), hb)
            for w in range(NW):
                gather1(hx1[:, w, :], hext1, dwin_sb[:, w:w + 1])

            # ---- phase B1 ----
            edge_phase(hext1, G1, 8, 8, uall, hx1)
            agg1 = finish(hx1, uall, 8, 8, b1_sb)
            ex1 = fp.tile([P, NW, 64], F32, tag="Uf")
            nc.scalar.activation(out=ex1[:], in_=agg1[:], func=AF.Exp)
            em = fp.tile([P, NW, 64], F32, tag="recip")
            nc.vector.tensor_scalar(out=em[:], in0=ex1[:], scalar1=-1.0,
                                    scalar2=0.0, op0=OP.add, op1=OP.min)
            nc.vector.scalar_tensor_tensor(
                out=h1act[:], in0=agg1[:], scalar=0.0, in1=em[:],
                op0=OP.max, op1=OP.add)

            # ---- phase A2 ----
            with (tc.tile_pool(name="a2", bufs=3) as a2,
                  tc.tile_pool(name="p2", bufs=1, space="PSUM") as p2):
                for w in range(NW):
                    hT_ps = p2.tile([64, P], F16, space="PSUM", tag="hT")
                    nc.tensor.transpose(out=hT_ps[:], in_=h1act[:, w, :],
                                        identity=ident_sb[:])
                    hT_sb = a2.tile([64, P], F16, tag="hT_sb")
                    nc.scalar.activation(out=hT_sb[:], in_=hT_ps[:],
                                         func=AF.Copy)
                    ps2 = p2.tile([P, W2O], F32, space="PSUM", tag="ps2")
                    nc.tensor.matmul(out=ps2[:], lhsT=hT_sb[:], rhs=w2_sb[:],
                                     start=True, stop=True)
                    nc.scalar.activation(out=hx2[:, w, :], in_=ps2[:],
                                         func=AF.Copy)
                dma_rows_out(h2_shard, 0, shard, hx2)
            nc.gpsimd.collective_compute(
                "AllGather", OP.bypass, replica_groups=[list(range(n_cores))],
                ins=[h2_shard[:]], outs=[hext2[:]])

            # ---- phase B2 ----
            edge_phase(hext2, G2, 1, 40, ual2, hx2)
            agg2 = finish(hx2, ual2, 1, 40, b2_sb)
            mx = fp.tile([P, NW, 1], F32, tag="den")
            nc.vector.reduce_max(out=mx[:], in_=agg2[:],
                                 axis=mybir.AxisListType.X)
            tm = fp.tile([P, NW, 40], F32, tag="Uf")
            nc.vector.tensor_sub(out=tm[:], in0=agg2[:],
                                 in1=mx[:, :, 0:1].to_broadcast([P, NW, 40]))
            q = fp.tile([P, NW, 40], F32, tag="exs")
            nc.scalar.activation(out=q[:], in_=tm[:], func=AF.Exp)
            s = fp.tile([P, NW, 1], F32, tag="es")
            nc.vector.reduce_sum(out=s[:], in_=q[:], axis=mybir.AxisListType.X)
            lsf = fp.tile([P, NW, 1], F32, tag="ls")
            nc.scalar.activation(out=lsf[:], in_=s[:], func=AF.Ln)
            o = fp.tile([P, NW, 40], F32, tag="recip")
            nc.vector.tensor_sub(out=o[:], in0=tm[:],
                                 in1=lsf[:, :, 0:1].to_broadcast([P, NW, 40]))
            dma_rows_out(out, 0, shard, o)

        for pool in (pd, pr, pu, fp, dfp, sp, rp, ohp, gp, cp):
            pool.release()

    nc.compile()
    return nc


_CACHE = {}


def _get_nc(T, reps=1):
    key = (T, reps)
    if key not in _CACHE:
        _CACHE[key] = build_kernel(N_NODES, T, reps=reps)
    return _CACHE[key]


def make_in_maps(x, edge_index, W1, a1_src, a1_dst, b1, W2, a2_src, a2_dst, b2,
                 T, N=None, n_cores=NC):
    N = N or N_NODES
    W_ext1, W_ext2 = _fold_params(W1, a1_src, a1_dst, W2, a2_src, a2_dst)
    src = np.asarray(edge_index[0]).astype(np.int64)
    dst = np.asarray(edge_index[1]).astype(np.int64)
    per_core, NW, NT = _prep_edges(src, dst, N, T, n_cores)
    shared = {
        "xT": np.ascontiguousarray(
            np.asarray(x, np.float32).T.astype(np.float16)),
        "W_ext1": W_ext1, "W_ext2": W_ext2,
        "b1m": np.tile(np.asarray(b1, np.float32)[None, :], (P, 1)),
        "b2m": np.tile(np.asarray(b2, np.float32)[None, :], (P, 1)),
        "iota_in": np.tile(np.arange(P, dtype=np.float16), (P, 1)),
        "iotac_in": np.arange(P, dtype=np.float32)[:, None],
        "ones_in": np.ones((1, P), np.float16),
        "ident_in": np.eye(P, dtype=np.float16),
    }
    return [dict(shared, sidx_in=si, doff_in=do, dofT_in=dt, dwin_in=dw)
            for (si, do, dt, dw) in per_core]


def required_T(edge_index, N=None, n_cores=NC):
    N = N or N_NODES
    dst = np.asarray(edge_index[1]).astype(np.int64)
    shard = N // n_cores
    maxt = 1
    for c in range(n_cores):
        ld = dst[(dst >= c * shard) & (dst < (c + 1) * shard)] - c * shard
        wc = np.bincount(ld >> 7, minlength=(shard + P - 1) // P)
        maxt = max(maxt, int(np.ceil(wc.max() / P)))
    return maxt


def kernel(x, edge_index, W1, a1_src, a1_dst, b1, W2, a2_src, a2_dst, b2,
           reps=1, nc_override=None):
    x = np.asarray(x, np.float32)
    edge_index = np.asarray(edge_index)
    args = [np.asarray(a, np.float32) for a in
            (W1, a1_src, a1_dst, b1, W2, a2_src, a2_dst, b2)]
    T = required_T(edge_index)
    in_maps = make_in_maps(x, edge_index, *args, T)
    nc = nc_override if nc_override is not None else _get_nc(T, reps)
    res = run_bass_kernel_spmd(nc, in_maps, list(range(NC)))
    return np.concatenate([res.results[c]["out"] for c in range(NC)], axis=0)
